# revision 34
# baseline (speedup 1.0000x reference)
"""BERT-base (12-layer) forward pass on 8 Trainium2 NeuronCores.

Strategy: data-parallel over batch (B=8 -> 1 sequence per core), no
collectives. Host casts weights to bf16 and folds each LayerNorm's gain
into the weight matrices that consume its output. On device the
projections (Q/K/V and FFN1) run on the *pre*-LayerNorm residual: the
mean correction enters the same PSUM accumulation as a rank-1 matmul
row, and the per-token 1/sigma is applied at PSUM evacuation. The
LayerNorm stats tail and row math of each sublayer are emitted inside
the *next* phase's first matmul chain so the in-order PE queue never
stalls behind them. Attention-output chains are level-interleaved in
groups of three so the softmax-normalize tail overlaps matmuls.

LayerNorm rstd uses Ln/Exp so the scalar engine only loads the
natural_log_exp and gelu table sets. The V projection is sequence-major
augmented with a ones column so the attention context matmul yields
softmax denominators for free.
"""
import sys
import os

if "/opt/trn_rl_repo" not in sys.path:
    sys.path.insert(0, "/opt/trn_rl_repo")

import numpy as np
import ml_dtypes

import concourse.bass as bass
from concourse import bacc
import concourse.tile as tile
from concourse import mybir
from concourse.bass_utils import run_bass_kernel_spmd
from concourse.masks import make_identity

F32 = mybir.dt.float32
BF16 = mybir.dt.bfloat16
INT32 = mybir.dt.int32
AF = mybir.ActivationFunctionType
ALU = mybir.AluOpType

B, S, H, NH, L, F = 8, 512, 768, 12, 12, 3072
V, TV, PP = 21128, 2, 512
DH = H // NH            # 64
P = 128
HT = H // P             # 6
FT = F // P             # 24
ST = S // P             # 4
EPS = 1e-12
NCORES = 8

NL = int(os.environ.get("BERT_KERNEL_LAYERS", str(L)))

_CACHE = {}

# param table columns (per layer, r6 layout [P, col])
PC_BAO = 0
PC_LN1G = 6
PC_LN1B = 12
PC_LN2G = 18
PC_LN2B = 24
PC_BIO = 30
NPC = 36

# rank-1 table columns: [2, 2H + H + F]; row 0 = -colsum(W_eff), row 1 = bias
RK_Q = 0
RK_K = H
RK_V = 2 * H
RK_I = 3 * H
RKW = 3 * H + F


def _build(has_bias=False):
    nc = bacc.Bacc("TRN2", target_bir_lowering=False, debug=False)

    WQKVO = nc.dram_tensor("wqkvo", [NL, 4, P, HT, H], BF16, kind="ExternalInput")
    WI = nc.dram_tensor("wi", [NL, P, HT, F], BF16, kind="ExternalInput")
    WIO = nc.dram_tensor("wio", [NL, P, FT, H], BF16, kind="ExternalInput")
    PARAMS = nc.dram_tensor("params", [NL, P, NPC], F32, kind="ExternalInput")
    RK1 = nc.dram_tensor("rk1", [NL, 2, RKW], BF16, kind="ExternalInput")
    TOK = nc.dram_tensor("tok", [V, H], F32, kind="ExternalInput")
    POSN = nc.dram_tensor("posn", [P, ST, H], F32, kind="ExternalInput")
    EMBR = nc.dram_tensor("embr", [P, 4, H], F32, kind="ExternalInput")
    IDS = nc.dram_tensor("ids", [P, ST], INT32, kind="ExternalInput")
    SEGF = nc.dram_tensor("segf", [P, ST], F32, kind="ExternalInput")
    MASKT = nc.dram_tensor("maskt", [P, ST], F32, kind="ExternalInput")
    OUT = nc.dram_tensor("out", [H, S], F32, kind="ExternalOutput")

    with tile.TileContext(nc) as tc:
        with (
            tc.tile_pool(name="const", bufs=1) as cpool,
            tc.tile_pool(name="act", bufs=1) as apool,
            tc.tile_pool(name="res", bufs=2) as rpool,
            tc.tile_pool(name="rows", bufs=1) as rows,
            tc.tile_pool(name="bcast", bufs=1) as bcp,
            tc.tile_pool(name="ep", bufs=2) as eppool,
            tc.tile_pool(name="pp", bufs=1, space="PSUM") as pp,
            tc.tile_pool(name="psc", bufs=2, space="PSUM") as psc,
            tc.tile_pool(name="pcx", bufs=2, space="PSUM") as pcx,
        ):
            # ---- persistent constants ----
            params = cpool.tile([P, NL, NPC], F32, tag="params")
            nc.sync.dma_start(params[:], PARAMS[:].rearrange("l p c -> p l c"))
            maskt = cpool.tile([P, ST], F32, tag="maskt")
            nc.sync.dma_start(maskt[:], MASKT[:])
            ones_b = cpool.tile([P, 1], BF16, tag="ones")
            nc.any.memset(ones_b[:], 1.0)
            epsc = cpool.tile([P, 1], F32, tag="epsc")
            nc.any.memset(epsc[:], EPS)
            ones_bh = cpool.tile([P, 1], BF16, tag="onesh")
            nc.any.memset(ones_bh[:], 1.0 / H)
            # layer-0 rows (input already normalized): mean = 0, sigma = 1
            mrow_bf0 = cpool.tile([1, S], BF16, tag="mrow0")
            nc.any.memset(mrow_bf0[:], 0.0)
            if has_bias:
                sigrow0 = cpool.tile([1, S], BF16, tag="sig0")
                nc.any.memset(sigrow0[:], 1.0)
            ident = cpool.tile([P, P], F32, tag="ident")
            make_identity(nc, ident)

            # ---- persistent activations ----
            qTb = apool.tile([P, HT, S], BF16, tag="qTb")
            kTb = apool.tile([P, HT, S], BF16, tag="kTb")
            ctxTb = apool.tile([P, HT, S], BF16, tag="ctxTb")
            v_aug = apool.tile([P, ST, NH * (DH + 1)], BF16, tag="v_aug")
            hTb = apool.tile([P, FT, S], BF16, tag="hTb")

            va_view = v_aug[:].rearrange("p st (h d) -> p st h d", d=DH + 1)
            nc.any.memset(va_view[:, :, :, DH : DH + 1], 1.0)

            # ============ embedding ============
            xT = rpool.tile([P, HT, S], F32, tag="pf")
            xTb = rpool.tile([P, HT, S], BF16, tag="pb")
            with tc.tile_pool(name="embp", bufs=1) as embp:
                posn = embp.tile([P, ST, H], F32, tag="posn")
                nc.sync.dma_start(posn[:], POSN[:])
                embr = embp.tile([P, 4, H], F32, tag="embr")
                nc.sync.dma_start(embr[:], EMBR[:])
                ids = embp.tile([P, ST], INT32, tag="ids")
                nc.sync.dma_start(ids[:], IDS[:])
                segf = embp.tile([P, ST], F32, tag="segf")
                nc.sync.dma_start(segf[:], SEGF[:])

                x0s = []
                for st in range(ST):
                    x0 = embp.tile([P, H], F32, tag="x0", bufs=4,
                                   name=f"x0_{st}")
                    nc.gpsimd.indirect_dma_start(
                        out=x0[:],
                        out_offset=None,
                        in_=TOK[:],
                        in_offset=bass.IndirectOffsetOnAxis(
                            ap=ids[:, st : st + 1], axis=0
                        ),
                    )
                    x0s.append(x0)
                for st in range(ST):
                    x0 = x0s[st]
                    tseg = embp.tile([P, H], F32, tag="tseg", bufs=2)
                    nc.scalar.activation(
                        tseg[:], embr[:, 1], AF.Copy, scale=segf[:, st : st + 1]
                    )
                    nc.vector.tensor_add(out=x0[:], in0=x0[:], in1=posn[:, st])
                    nc.vector.tensor_add(out=x0[:], in0=x0[:], in1=embr[:, 0])
                    nc.vector.tensor_add(out=x0[:], in0=x0[:], in1=tseg[:])
                    s1 = embp.tile([P, 1], F32, tag="s1", bufs=2)
                    nc.vector.reduce_sum(s1[:], x0[:], axis=mybir.AxisListType.X)
                    sqs = embp.tile([P, H], F32, tag="sqs", bufs=2)
                    ssq = embp.tile([P, 1], F32, tag="ssq", bufs=2)
                    nc.scalar.activation(sqs[:], x0[:], AF.Square, accum_out=ssq[:])
                    mean = embp.tile([P, 1], F32, tag="mean", bufs=2)
                    nc.any.tensor_scalar_mul(mean[:], s1[:], 1.0 / H)
                    msq = embp.tile([P, 1], F32, tag="msq", bufs=2)
                    nc.any.tensor_scalar_mul(msq[:], ssq[:], 1.0 / H)
                    var = embp.tile([P, 1], F32, tag="var", bufs=2)
                    nc.vector.tensor_tensor(var[:], mean[:], mean[:], ALU.mult)
                    nc.vector.tensor_tensor(var[:], msq[:], var[:], ALU.subtract)
                    rstd = embp.tile([P, 1], F32, tag="rstd", bufs=2)
                    nc.scalar.activation(rstd[:], var[:], AF.Sqrt, bias=epsc[:])
                    nc.vector.reciprocal_approx_fast(rstd[:], rstd[:])
                    negmr = embp.tile([P, 1], F32, tag="negmr", bufs=2)
                    nc.vector.tensor_tensor(negmr[:], mean[:], rstd[:], ALU.mult)
                    nc.any.tensor_scalar_mul(negmr[:], negmr[:], -1.0)
                    nc.scalar.activation(
                        x0[:], x0[:], AF.Identity, bias=negmr[:], scale=rstd[:]
                    )
                    nc.vector.tensor_tensor(x0[:], x0[:], embr[:, 2], ALU.mult)
                    nc.vector.tensor_tensor(x0[:], x0[:], embr[:, 3], ALU.add)
                    for ht in range(HT):
                        pt = pp.tile([P, S], F32, tag="proj", bufs=3)
                        nc.tensor.transpose(
                            pt[:, :P], x0[:, P * ht : P * (ht + 1)], ident[:]
                        )
                        nc.vector.tensor_copy(
                            xT[:, ht, P * st : P * (st + 1)], pt[:, :P]
                        )
                        nc.scalar.copy(xTb[:, ht, P * st : P * (st + 1)], pt[:, :P])

            # ============ transformer layers ============
            st8 = {
                "xF": xT, "xB": xTb,
                "mrow": mrow_bf0,
                "sigrow": sigrow0 if has_bias else None,
                "rb": None,           # [P,S] rstd broadcast (None = ones)
                "rbmb2": None, "rowbuf2": None,
            }
            deferred = []             # closures to emit inside next phase

            def flush():
                while deferred:
                    deferred.pop(0)()

            def rows_chain(pst):
                """Emit row math for one LN. Returns (rbmb, rowbuf, mrow_bf,
                sigrow_bf)."""
                rowbuf = rows.tile([1, 2 * S], F32, tag="rowbuf", bufs=2)
                mrow = rowbuf[0:1, S : 2 * S]
                nc.vector.tensor_copy(mrow, pst[0:1, :])
                mrow_bf = rows.tile([1, S], BF16, tag="mrowb", bufs=2)
                nc.vector.tensor_copy(mrow_bf[:], pst[0:1, :])
                m2 = rows.tile([1, S], F32, tag="m2", bufs=1)
                nc.vector.tensor_tensor(m2[:], mrow, mrow, ALU.mult)
                nc.vector.tensor_tensor(m2[:], pst[64:65, :], m2[:], ALU.subtract)
                nc.scalar.activation(
                    rowbuf[0:1, 0:S], m2[:], AF.Sqrt, bias=epsc[0:1, :]
                )
                sigrow_bf = None
                if has_bias:
                    sigrow_bf = rows.tile([1, S], BF16, tag="sigb", bufs=2)
                    nc.vector.tensor_copy(sigrow_bf[:], rowbuf[0:1, 0:S])
                nc.vector.reciprocal_approx_fast(
                    rowbuf[0:1, 0:S], rowbuf[0:1, 0:S]
                )
                nc.vector.tensor_tensor(mrow, mrow, rowbuf[0:1, 0:S], ALU.mult)
                rbmb = bcp.tile([P, 2 * S], F32, tag="rbmb", bufs=2)
                nc.gpsimd.partition_broadcast(rbmb[:], rowbuf[:])
                return rbmb, rowbuf, mrow_bf, sigrow_bf

            def rank1(ps_slice, lhs_w, mrow, sigrow, rhs_w, flip, stop=True):
                """Accumulate the mean (and bias) rank-1 rows into ps.
                flip=False: lhsT = table chunk, rhs = device row (Q/K/FFN1).
                flip=True:  lhsT = device row chunk, rhs = table (V)."""
                if flip:
                    nc.tensor.matmul(ps_slice, mrow, lhs_w, start=False,
                                     stop=stop and not has_bias)
                    if has_bias:
                        nc.tensor.matmul(ps_slice, sigrow, rhs_w,
                                         start=False, stop=stop)
                else:
                    nc.tensor.matmul(ps_slice, lhs_w, mrow, start=False,
                                     stop=stop and not has_bias)
                    if has_bias:
                        nc.tensor.matmul(ps_slice, rhs_w, sigrow,
                                         start=False, stop=stop)

            def stats_pair(pst, j, preB, sqj, last):
                nc.tensor.matmul(
                    pst[0:1, :], ones_bh[:], preB[:, j],
                    start=(j == 0), stop=last,
                )
                nc.tensor.matmul(
                    pst[64:65, :], ones_b[:], sqj[:],
                    start=(j == 0), stop=last,
                )

            def materialize_ops(preF, rbmb, gcol, bcol):
                """LN output materialization in place, as a list of op
                closures (DVE mult/sub + ACT affine) so callers can
                interleave them into phases with engine slack. GpSimd is
                kept broadcast-only: mixing op families there causes
                ~10us LIBRARY_RELOAD stalls."""
                rb = rbmb[:, 0:S]
                mb = rbmb[:, S : 2 * S]
                ops = []
                for kt in range(HT):
                    ops.append(lambda kt=kt: nc.vector.tensor_tensor(
                        preF[:, kt], preF[:, kt], rb, ALU.mult))
                    ops.append(lambda kt=kt: nc.vector.tensor_tensor(
                        preF[:, kt], preF[:, kt], mb, ALU.subtract))
                    ops.append(lambda kt=kt: nc.scalar.activation(
                        preF[:, kt], preF[:, kt], AF.Identity,
                        bias=bcol[:, kt : kt + 1], scale=gcol[:, kt : kt + 1]))
                return ops

            with (
                tc.tile_pool(name="wq", bufs=2) as wqpool,
                tc.tile_pool(name="wf", bufs=2) as wfpool,
                tc.tile_pool(name="wo", bufs=1) as wopool,
                tc.tile_pool(name="rk", bufs=1) as rkpool,
            ):
                wq_next = wqpool.tile([P, HT, H], BF16, tag="wqk")
                nc.sync.dma_start(wq_next[:], WQKVO[0, 0])
                wk_next = wqpool.tile([P, HT, H], BF16, tag="wqk")
                nc.sync.dma_start(wk_next[:], WQKVO[0, 1])
                rk_next = rkpool.tile([2, RKW], BF16, tag="rk1")
                nc.sync.dma_start(rk_next[:], RK1[0])

                for l in range(NL):
                    pb = params[:, l, :]
                    xB_in = st8["xB"]
                    xF_in = st8["xF"]

                    scope = nc.named_scope(f"qk_{l}"); scope.__enter__()
                    rk1t = rk_next

                    # ---- Q, K projections from pre-LN residual ----
                    for pi, dst in ((0, qTb), (1, kTb)):
                        w = wq_next if pi == 0 else wk_next
                        rkbase = RK_Q if pi == 0 else RK_K
                        pend = []
                        for nt in range(HT):
                            ps = pp.tile([P, S], F32, tag="proj", bufs=3)
                            for kt in range(HT):
                                nc.tensor.matmul(
                                    ps[:], w[:, kt, P * nt : P * (nt + 1)],
                                    xB_in[:, kt],
                                    start=(kt == 0), stop=False,
                                )
                            if pi == 0 and nt == 0:
                                # previous LN2 stats tail + row math + bcast
                                flush()
                            pend.append((nt, ps))
                            if len(pend) == 3:
                                j, pj = pend.pop(0)
                                rank1(
                                    pj[:],
                                    rk1t[0:1, rkbase + P * j : rkbase + P * (j + 1)],
                                    st8["mrow"][:], st8["sigrow"],
                                    rk1t[1:2, rkbase + P * j : rkbase + P * (j + 1)],
                                    flip=False,
                                )
                                if st8["rb"] is None:
                                    nc.vector.tensor_copy(dst[:, j], pj[:])
                                else:
                                    nc.vector.tensor_tensor(
                                        dst[:, j], pj[:], st8["rb"], ALU.mult
                                    )
                        for j, pj in pend:
                            rank1(
                                pj[:],
                                rk1t[0:1, rkbase + P * j : rkbase + P * (j + 1)],
                                st8["mrow"][:], st8["sigrow"],
                                rk1t[1:2, rkbase + P * j : rkbase + P * (j + 1)],
                                flip=False,
                            )
                            if st8["rb"] is None:
                                nc.vector.tensor_copy(dst[:, j], pj[:])
                            else:
                                nc.vector.tensor_tensor(
                                    dst[:, j], pj[:], st8["rb"], ALU.mult
                                )
                        if pi == 0:
                            # rcol (per-token rstd column for V evac) + xT
                            # materialization, off the hot queue head
                            if l > 0:
                                rcol = rows.tile([P, ST], F32, tag="rcol", bufs=2)
                                rps = pp.tile([P, S], F32, tag="proj", bufs=3)
                                for st in range(ST):
                                    nc.tensor.transpose(
                                        rps[:, st : st + 1],
                                        st8["rowbuf2"][0:1, P * st : P * (st + 1)],
                                        ident[0:1, 0:1],
                                    )
                                nc.vector.tensor_copy(rcol[:], rps[:, 0:ST])
                                rcol_in = rcol
                                mat_x = materialize_ops(
                                    st8["xF"], st8["rbmb2"],
                                    params[:, l - 1, PC_LN2G : PC_LN2G + 6],
                                    params[:, l - 1, PC_LN2B : PC_LN2B + 6],
                                )
                            else:
                                rcol_in = None
                                mat_x = []
                    xT_res = st8["xF"]
                    scope.__exit__(None, None, None)

                    scope = nc.named_scope(f"v_{l}"); scope.__enter__()
                    # ---- V projection (seq-major, augmented ones column) ----
                    wv = wqpool.tile([P, HT, H], BF16, tag="wqk")
                    nc.sync.dma_start(wv[:], WQKVO[l, 2])
                    wao = wqpool.tile([P, HT, H], BF16, tag="wqk")
                    nc.sync.dma_start(wao[:], WQKVO[l, 3])
                    for st in range(ST):
                        for half in range(2):
                            ps = pp.tile([P, S], F32, tag="proj", bufs=3)
                            for kt in range(HT):
                                nc.tensor.matmul(
                                    ps[:, :384],
                                    xB_in[:, kt, P * st : P * (st + 1)],
                                    wv[:, kt, 384 * half : 384 * (half + 1)],
                                    start=(kt == 0), stop=False,
                                )
                            rank1(
                                ps[:, :384],
                                rk1t[0:1, RK_V + 384 * half : RK_V + 384 * (half + 1)],
                                st8["mrow"][0:1, P * st : P * (st + 1)],
                                (st8["sigrow"][0:1, P * st : P * (st + 1)]
                                 if has_bias else None),
                                rk1t[1:2, RK_V + 384 * half : RK_V + 384 * (half + 1)],
                                flip=True,
                            )
                            dst3 = va_view[:, st, 6 * half : 6 * (half + 1), 0:DH]
                            src3 = ps[:, :384].rearrange("p (h d) -> p h d", d=DH)
                            if rcol_in is None:
                                nc.scalar.activation(dst3, src3, AF.Identity)
                            else:
                                nc.scalar.activation(
                                    dst3, src3, AF.Identity,
                                    scale=rcol_in[:, st : st + 1],
                                )
                    wih0 = wfpool.tile([P, HT, F // 4], BF16, tag="wi")
                    nc.sync.dma_start(wih0[:], WI[l][:, :, 0 : F // 4])
                    wih1 = wfpool.tile([P, HT, F // 4], BF16, tag="wi")
                    nc.sync.dma_start(wih1[:], WI[l][:, :, F // 4 : F // 2])
                    scope.__exit__(None, None, None)

                    scope = nc.named_scope(f"attn_{l}"); scope.__enter__()
                    # ---- attention ----
                    for ht in range(HT):
                        expt_a = eppool.tile([P, ST, S], BF16, tag="expt")
                        expt_b = eppool.tile([P, ST, S], BF16, tag="expt")
                        expts = (expt_a, expt_b)
                        for kti in range(ST):
                            pss = []
                            for sub in range(2):
                                base = 64 * sub
                                pssc = psc.tile([P, S], F32, tag="sc")
                                nc.tensor.matmul(
                                    pssc[:],
                                    kTb[base : base + DH, ht, P * kti : P * (kti + 1)],
                                    qTb[base : base + DH, ht, :],
                                    start=True, stop=True,
                                )
                                pss.append(pssc)
                            for sub in range(2):
                                nc.scalar.activation(
                                    expts[sub][:, kti], pss[sub][:], AF.Exp,
                                    scale=0.125, bias=maskt[:, kti : kti + 1],
                                )
                        for sub in range(2):
                            h = 2 * ht + sub
                            base = 64 * sub
                            expt = expts[sub]
                            ppc = pcx.tile([DH + 1, S], F32, tag="cx")
                            for kti in range(ST):
                                nc.tensor.matmul(
                                    ppc[:],
                                    v_aug[:, kti, (DH + 1) * h : (DH + 1) * (h + 1)],
                                    expt[:, kti],
                                    start=(kti == 0), stop=(kti == ST - 1),
                                )
                            srow = rows.tile([1, S], F32, tag="srow", bufs=1)
                            nc.vector.tensor_copy(srow[:], ppc[DH : DH + 1, :])
                            recrow = rows.tile([1, S], F32, tag="recrow", bufs=1)
                            nc.vector.reciprocal_approx_fast(recrow[:], srow[:])
                            recb = bcp.tile([DH, S], F32, tag="recb", bufs=1)
                            nc.gpsimd.partition_broadcast(recb[:], recrow[:])
                            nc.vector.tensor_tensor(
                                ctxTb[base : base + DH, ht, :], ppc[:DH, :], recb[:],
                                ALU.mult,
                            )
                            for _ in range(3):
                                if mat_x:
                                    mat_x.pop(0)()
                    while mat_x:
                        mat_x.pop(0)()
                    scope.__exit__(None, None, None)

                    scope = nc.named_scope(f"ao_{l}"); scope.__enter__()
                    # ---- attention output + residual (level-interleaved) ----
                    pre1F = rpool.tile([P, HT, S], F32, tag="pf")
                    pre1B = rpool.tile([P, HT, S], BF16, tag="pb")
                    pst1 = pp.tile([P, S], F32, tag="stat", bufs=1)
                    sqs1 = {}
                    for nt in range(HT):
                        ps = pp.tile([P, S], F32, tag="proj", bufs=3)
                        for kt in range(HT):
                            nc.tensor.matmul(
                                ps[:], wao[:, kt, P * nt : P * (nt + 1)],
                                ctxTb[:, kt],
                                start=(kt == 0), stop=(kt == HT - 1),
                            )
                        nc.vector.scalar_tensor_tensor(
                            pre1F[:, nt], ps[:],
                            pb[:, PC_BAO + nt : PC_BAO + nt + 1],
                            xT_res[:, nt], ALU.add, ALU.add,
                        )
                        nc.scalar.copy(pre1B[:, nt], pre1F[:, nt])
                        sq1 = bcp.tile([P, S], BF16, tag="sq1", bufs=2)
                        nc.vector.scalar_tensor_tensor(
                            sq1[:], pre1F[:, nt], 1.0 / H, pre1F[:, nt],
                            ALU.mult, ALU.mult,
                        )
                        sqs1[nt] = sq1
                        if nt >= 1:
                            stats_pair(pst1, nt - 1, pre1B, sqs1[nt - 1], False)

                    def defer_ln1(pst1=pst1, pre1B=pre1B, sqs1=sqs1):
                        stats_pair(pst1, HT - 1, pre1B, sqs1[HT - 1], True)
                        rbmb1, _, mrow1, sig1 = rows_chain(pst1)
                        st8["rbmb1"] = rbmb1
                        st8["mrow1"] = mrow1
                        st8["sig1"] = sig1
                    deferred.append(defer_ln1)
                    scope.__exit__(None, None, None)

                    scope = nc.named_scope(f"ffn1_{l}"); scope.__enter__()
                    # ---- FFN1 (gelu) from pre-LN residual ----
                    wio = wopool.tile([P, FT, H], BF16, tag="wio")
                    nc.sync.dma_start(wio[:], WIO[l])
                    if l + 1 < NL:
                        wq_next = wqpool.tile([P, HT, H], BF16, tag="wqk")
                        nc.sync.dma_start(wq_next[:], WQKVO[l + 1, 0])
                    pend = []
                    for quarter in range(4):
                        if quarter < 2:
                            wih = (wih0, wih1)[quarter]
                        else:
                            wih = wfpool.tile([P, HT, F // 4], BF16, tag="wi")
                            nc.sync.dma_start(
                                wih[:],
                                WI[l][:, :, (F // 4) * quarter : (F // 4) * (quarter + 1)],
                            )
                        for ntl in range(6):
                            nt = 6 * quarter + ntl
                            if nt % 4 == 3:
                                ps = pp.tile([P, S], F32, tag="stat", bufs=1,
                                             name=f"f1st{l}_{nt}")
                            else:
                                ps = pp.tile([P, S], F32, tag="proj", bufs=3,
                                             name=f"f1ps{l}_{nt}")
                            for kt in range(HT):
                                nc.tensor.matmul(
                                    ps[:], wih[:, kt, P * ntl : P * (ntl + 1)],
                                    pre1B[:, kt],
                                    start=(kt == 0), stop=False,
                                )
                            if nt == 0:
                                flush()   # LN1 stats tail + row math
                                mat_a = materialize_ops(
                                    pre1F, st8["rbmb1"],
                                    pb[:, PC_LN1G : PC_LN1G + 6],
                                    pb[:, PC_LN1B : PC_LN1B + 6],
                                )
                            pend.append((nt, ps))
                            if len(pend) == 3:
                                j, pj = pend.pop(0)
                                rank1(
                                    pj[:],
                                    rk1t[0:1, RK_I + P * j : RK_I + P * (j + 1)],
                                    st8["mrow1"][:], st8["sig1"],
                                    rk1t[1:2, RK_I + P * j : RK_I + P * (j + 1)],
                                    flip=False,
                                )
                                nc.vector.tensor_tensor(
                                    pj[:], pj[:], st8["rbmb1"][:, 0:S], ALU.mult
                                )
                                nc.scalar.activation(hTb[:, j], pj[:], AF.Gelu)
                    for j, pj in pend:
                        rank1(
                            pj[:], rk1t[0:1, RK_I + P * j : RK_I + P * (j + 1)],
                            st8["mrow1"][:], st8["sig1"],
                            rk1t[1:2, RK_I + P * j : RK_I + P * (j + 1)],
                            flip=False,
                        )
                        nc.vector.tensor_tensor(
                            pj[:], pj[:], st8["rbmb1"][:, 0:S], ALU.mult
                        )
                        nc.scalar.activation(hTb[:, j], pj[:], AF.Gelu)
                    while mat_a:
                        mat_a.pop(0)()
                    scope.__exit__(None, None, None)

                    scope = nc.named_scope(f"ffn2_{l}"); scope.__enter__()
                    # ---- FFN2 + residual; stats for LN2 ----
                    if l + 1 < NL:
                        wk_next = wqpool.tile([P, HT, H], BF16, tag="wqk")
                        nc.sync.dma_start(wk_next[:], WQKVO[l + 1, 1])
                        rk_next = rkpool.tile([2, RKW], BF16, tag="rk1")
                        nc.sync.dma_start(rk_next[:], RK1[l + 1])
                    pre2F = rpool.tile([P, HT, S], F32, tag="pf")
                    pre2B = rpool.tile([P, HT, S], BF16, tag="pb")
                    pst2 = pp.tile([P, S], F32, tag="stat", bufs=1)
                    sqs2 = {}
                    for nt in range(HT):
                        ps = pp.tile([P, S], F32, tag="proj", bufs=3)
                        for kt in range(FT):
                            nc.tensor.matmul(
                                ps[:], wio[:, kt, P * nt : P * (nt + 1)], hTb[:, kt],
                                start=(kt == 0), stop=(kt == FT - 1),
                            )
                        nc.vector.scalar_tensor_tensor(
                            pre2F[:, nt], ps[:], pb[:, PC_BIO + nt : PC_BIO + nt + 1],
                            pre1F[:, nt], ALU.add, ALU.add,
                        )
                        nc.scalar.copy(pre2B[:, nt], pre2F[:, nt])
                        sq1 = bcp.tile([P, S], BF16, tag="sq1", bufs=2)
                        nc.vector.scalar_tensor_tensor(
                            sq1[:], pre2F[:, nt], 1.0 / H, pre2F[:, nt],
                            ALU.mult, ALU.mult,
                        )
                        sqs2[nt] = sq1
                        if nt >= 1:
                            stats_pair(pst2, nt - 1, pre2B, sqs2[nt - 1], False)

                    def defer_ln2(pst2=pst2, pre2B=pre2B, sqs2=sqs2):
                        stats_pair(pst2, HT - 1, pre2B, sqs2[HT - 1], True)
                        rbmb2, rowbuf2, mrow2, sig2 = rows_chain(pst2)
                        st8["rbmb2"] = rbmb2
                        st8["rowbuf2"] = rowbuf2
                        st8["mrow"] = mrow2
                        st8["sigrow"] = sig2
                        st8["rb"] = rbmb2[:, 0:S]
                    deferred.append(defer_ln2)
                    st8["xF"] = pre2F
                    st8["xB"] = pre2B
                    scope.__exit__(None, None, None)

                # ============ final LN + output ============
                flush()
                for op in materialize_ops(
                    st8["xF"], st8["rbmb2"],
                    params[:, NL - 1, PC_LN2G : PC_LN2G + 6],
                    params[:, NL - 1, PC_LN2B : PC_LN2B + 6],
                ):
                    op()
                nc.sync.dma_start(
                    OUT[:].rearrange("(ht p) s -> p ht s", p=P), st8["xF"][:]
                )

    nc.compile()
    return nc


def _r6(v):
    return np.ascontiguousarray(v.reshape(6, P).T)


def _prep_shared(inputs):
    bf = ml_dtypes.bfloat16
    f32 = np.float32
    ln2_g = np.asarray(inputs["ln2_g"], f32)
    ln1_g = np.asarray(inputs["ln1_g"], f32)

    has_bias = any(
        float(np.abs(np.asarray(inputs[k], f32)).max()) > 0
        for k in ("bq", "bk", "bv", "bi")
    )

    wqkvo = np.empty((NL, 4, P, HT, H), dtype=bf)
    rk1 = np.zeros((NL, 2, RKW), dtype=f32)
    for l in range(NL):
        g_in = ln2_g[l - 1] if l > 0 else np.ones(H, f32)
        for pi, name in enumerate(("Wq", "Wk", "Wv", "Wao")):
            w = np.asarray(inputs[name][l], f32)
            if pi < 3:
                w = w * g_in[:, None]
                base = (RK_Q, RK_K, RK_V)[pi]
                rk1[l, 0, base : base + H] = -w.sum(axis=0)
                rk1[l, 1, base : base + H] = np.asarray(
                    inputs[("bq", "bk", "bv")[pi]][l], f32
                )
            wqkvo[l, pi] = w.reshape(HT, P, H).transpose(1, 0, 2).astype(bf)
    wi = np.empty((NL, P, HT, F), dtype=bf)
    wio = np.empty((NL, P, FT, H), dtype=bf)
    for l in range(NL):
        w = np.asarray(inputs["Wi"][l], f32) * ln1_g[l][:, None]
        rk1[l, 0, RK_I : RK_I + F] = -w.sum(axis=0)
        rk1[l, 1, RK_I : RK_I + F] = np.asarray(inputs["bi"][l], f32)
        wi[l] = w.reshape(HT, P, F).transpose(1, 0, 2).astype(bf)
        wio[l] = (
            np.asarray(inputs["Wio"][l], f32)
            .reshape(FT, P, H).transpose(1, 0, 2).astype(bf)
        )
    params = np.zeros((NL, P, NPC), dtype=f32)
    for l in range(NL):
        params[l, :, PC_BAO : PC_BAO + 6] = _r6(np.asarray(inputs["bao"][l], f32))
        params[l, :, PC_LN1G : PC_LN1G + 6] = _r6(ln1_g[l])
        params[l, :, PC_LN1B : PC_LN1B + 6] = _r6(np.asarray(inputs["ln1_b"][l], f32))
        params[l, :, PC_LN2G : PC_LN2G + 6] = _r6(ln2_g[l])
        params[l, :, PC_LN2B : PC_LN2B + 6] = _r6(np.asarray(inputs["ln2_b"][l], f32))
        params[l, :, PC_BIO : PC_BIO + 6] = _r6(np.asarray(inputs["bio"][l], f32))
    tok = np.ascontiguousarray(np.asarray(inputs["tok_emb"], f32))
    posn = np.ascontiguousarray(
        np.asarray(inputs["pos_emb"], f32)[:S]
        .reshape(ST, P, H).transpose(1, 0, 2)
    )
    te = np.asarray(inputs["type_emb"], f32)
    embr = np.empty((P, 4, H), dtype=f32)
    embr[:, 0] = te[0]
    embr[:, 1] = te[1] - te[0]
    embr[:, 2] = np.asarray(inputs["emb_g"], f32)
    embr[:, 3] = np.asarray(inputs["emb_b"], f32)
    return {
        "wqkvo": wqkvo, "wi": wi, "wio": wio, "params": params,
        "rk1": rk1.astype(bf), "tok": tok, "posn": posn, "embr": embr,
    }, has_bias


def kernel(**inputs):
    shared, has_bias = _prep_shared(inputs)
    key = ("nc", has_bias)
    if key not in _CACHE:
        _CACHE[key] = _build(has_bias=has_bias)
    nc = _CACHE[key]

    ids_full = np.asarray(inputs["input_ids"], np.int32)
    seg_full = np.asarray(inputs["segment_ids"], np.int32)
    mask_full = np.asarray(inputs["attention_mask"], np.float32)

    in_maps = []
    for c in range(NCORES):
        m = dict(shared)
        m["ids"] = np.ascontiguousarray(ids_full[c].reshape(ST, P).T)
        m["segf"] = np.ascontiguousarray(
            seg_full[c].astype(np.float32).reshape(ST, P).T
        )
        mrow = (1.0 - mask_full[c, 0, 0]) * -10000.0
        m["maskt"] = np.ascontiguousarray(mrow.reshape(ST, P).T)
        in_maps.append(m)

    res = run_bass_kernel_spmd(nc, in_maps, core_ids=list(range(NCORES)))
    out = np.empty((B, S, H), dtype=np.float32)
    for c in range(NCORES):
        out[c] = res.results[c]["out"].T
    return out
